# revision 1
# baseline (speedup 1.0000x reference)
"""Multi-head self-attention block (B=4, N=4096, C=384, H=6, D=64) on 8 TRN2
NeuronCores.

Sharding: core c handles batch b = c // 2 and query rows
[(c % 2) * 2048, (c % 2 + 1) * 2048).  Fully data-parallel SPMD — the per-core
program is identical; only the input data differs.  No collectives.

Per-core dataflow (everything "transposed" so contractions sit on partitions):
  - K^T, Q^T tiles [head-dim on partitions, 2 heads per 128-partition chunk],
    V natural with an appended ones-column (so the AV matmul also produces the
    softmax denominator Z).
  - Attention streams over 32 key tiles per (query-512-block, head-pair):
    the two heads' S^T = K @ Q^T matmuls run concurrently on disjoint PE row
    groups into one [128, 1024] PSUM tile, one exp on ScalarE covers both
    (scale fused), AV accumulates [65, 512] per head in PSUM (row 64 = Z).
    Software-pipelined so ScalarE (the bottleneck engine) never waits.
  - QKV projection work that isn't needed immediately is drained one unit at
    a time into the attention stream (PE in-order would otherwise serialize
    the whole projection phase before the first exp).
  - Normalize with 1/Z (DMA-broadcast via a DRAM bounce), project with
    host-rearranged w_proj (K=64 chunks), bias via a K=1 matmul with ones.
  - Output written as y^T [384, 2048]; the host transposes back.
"""

import numpy as np
import ml_dtypes

import concourse.bass as bass
import concourse.tile as tile
import concourse.mybir as mybir
from concourse import bacc
from concourse.bass_utils import run_bass_kernel_spmd

# Problem dims (hardcoded per contract)
B, N, C = 4, 4096, 384
H, D = 6, 64
SCALE = D**-0.5
NCORES = 8
NQ = N // 2  # queries per core
QB = 512  # query block (one PSUM bank of AV accumulation per head)
NQB = NQ // QB  # 4
KT = 128  # key tile
NKT = N // KT  # 32
F32 = mybir.dt.float32
# 16-bit compute dtype: float16 streams at full PE rate (like bf16)
# but carries a 10-bit mantissa — ~8x lower rounding error.
BF16 = mybir.dt.float16
BF16_NP = np.float16


def _build():
    nc = bacc.Bacc(None, target_bir_lowering=False)

    xt = nc.dram_tensor("xt", [3, 128, N], BF16, kind="ExternalInput")
    xq = nc.dram_tensor("xq", [3, 128, NQ], BF16, kind="ExternalInput")
    wqkv = nc.dram_tensor("wqkv", [3, 128, 3 * C], BF16, kind="ExternalInput")
    wproj = nc.dram_tensor("wproj", [H, D, C], BF16, kind="ExternalInput")
    bproj = nc.dram_tensor("bproj", [1, C], BF16, kind="ExternalInput")
    out_t = nc.dram_tensor("out_t", [3, 128, NQ], F32, kind="ExternalOutput")

    with tile.TileContext(nc) as tc:
        with (
            tc.tile_pool(name="persist", bufs=1) as persist,
            tc.tile_pool(name="ppool", bufs=4) as ppool,
            tc.tile_pool(name="zpool", bufs=6) as zpool,
            tc.tile_pool(name="ypool", bufs=3) as ypool,
            tc.tile_pool(name="dpool", bufs=4, space="DRAM") as dpool,
            tc.tile_pool(name="spool", bufs=2, space="PSUM") as spool,
            tc.tile_pool(name="opool", bufs=4, space="PSUM") as opool,
        ):
            # ---- persistent SBUF tensors ----
            xt_sb = [persist.tile([128, N], BF16, tag=f"xt{c}", name=f"xt_sb{c}") for c in range(3)]
            xq_sb = [persist.tile([128, NQ], BF16, tag=f"xq{c}", name=f"xq_sb{c}") for c in range(3)]
            wq_sb = persist.tile([128, 3, 3 * C], BF16, tag="wq")
            wp_sb = persist.tile([D, H, C], BF16, tag="wp")
            bp_sb = persist.tile([1, C], BF16, tag="bp")
            ones_sb = persist.tile([1, QB], BF16, tag="ones")
            kt_sb = [persist.tile([128, N], BF16, tag=f"kt{j}", name=f"kt_sb{j}") for j in range(3)]
            qt_sb = [persist.tile([128, NQ], BF16, tag=f"qt{j}", name=f"qt_sb{j}") for j in range(3)]
            vp_sb = [
                persist.tile([128, H * (D + 1)], BF16, tag=f"vp{k}", name=f"vp_sb{k}")
                for k in range(NKT)
            ]
            at_sb = [persist.tile([D, NQ], BF16, tag=f"at{h}", name=f"at_sb{h}") for h in range(H)]

            # ---- input DMAs, ordered so the first QK matmuls start ASAP ----
            for c in range(3):
                nc.sync.dma_start(
                    out=wq_sb[:, c, 0 : 2 * C], in_=wqkv[c, :, 0 : 2 * C]
                )
            for c in range(3):
                nc.scalar.dma_start(
                    out=xt_sb[c][:, 0:1024], in_=xt[c, :, 0:1024]
                )
            for c in range(3):
                nc.gpsimd.dma_start(out=xq_sb[c][:, 0:512], in_=xq[c, :, 0:512])
            for c in range(3):
                nc.sync.dma_start(
                    out=wq_sb[:, c, 2 * C : 3 * C], in_=wqkv[c, :, 2 * C : 3 * C]
                )
            for t in range(1, 4):
                for c in range(3):
                    nc.gpsimd.dma_start(
                        out=xt_sb[c][:, t * 1024 : (t + 1) * 1024],
                        in_=xt[c, :, t * 1024 : (t + 1) * 1024],
                    )
            for c in range(3):
                nc.gpsimd.dma_start(out=xq_sb[c][:, 512:NQ], in_=xq[c, :, 512:NQ])
            nc.gpsimd.dma_start(out=wp_sb, in_=wproj.rearrange("h d c -> d h c"))
            nc.gpsimd.dma_start(out=bp_sb, in_=bproj[:, :])
            nc.vector.memset(ones_sb, 1.0)
            ones_f32 = persist.tile([D + 1, D], F32, tag="ones_f32")
            nc.vector.memset(ones_f32[D : D + 1, :], 1.0)

            # ---- QKV projection units (share the 1-bank opool slots) ----
            def v_unit(k):
                vps = opool.tile([128, C], F32, tag="oacc", name=f"vps{k}")
                for c in range(3):
                    nc.tensor.matmul(
                        vps,
                        xt_sb[c][:, k * KT : (k + 1) * KT],
                        wq_sb[:, c, 2 * C : 3 * C],
                        start=(c == 0),
                        stop=(c == 2),
                    )
                dst = vp_sb[k].rearrange("p (h e) -> p h e", e=D + 1)
                nc.vector.tensor_copy(
                    out=dst[:, :, 0:D], in_=vps.rearrange("p (h d) -> p h d", d=D)
                )
                nc.vector.memset(dst[:, :, D : D + 1], 1.0)

            def kq_unit(kind, j, t):
                kq = opool.tile([128, 512], F32, tag="oacc", name=f"kq{kind}{j}_{t}")
                coff = C + j * 128 if kind == "k" else j * 128
                src = xt_sb if kind == "k" else xq_sb
                dstt = kt_sb[j] if kind == "k" else qt_sb[j]
                for c in range(3):
                    nc.tensor.matmul(
                        kq,
                        wq_sb[:, c, coff : coff + 128],
                        src[c][:, t * 512 : (t + 1) * 512],
                        start=(c == 0),
                        stop=(c == 2),
                    )
                nc.vector.tensor_copy(out=dstt[:, t * 512 : (t + 1) * 512], in_=kq)

            # upfront: only head pair 0's first K/Q tiles; ALL other QKV
            # work (including V) drains into the attention stream.  PE is
            # in-order, so emission order guarantees each AV sees its V tile.
            kq_unit("k", 0, 0)
            kq_unit("q", 0, 0)

            units = []
            ks = [("k", 0, t) for t in range(1, N // 512)]
            for k in range(NKT):
                units.append(("v", k, 0))
                if k % 2 == 1 and ks:
                    units.append(ks.pop(0))
            units.extend(ks)
            for j in (1, 2):
                units.append(("k", j, 0))
                units.append(("q", j, 0))
                for t in range(1, N // 512):
                    units.append(("k", j, t))
                for t in range(1, NQ // 512):
                    units.append(("q", j, t))
            for t in range(1, NQ // 512):
                units.append(("q", 0, t))
            units.reverse()  # pop() from the end

            def emit_unit(u):
                if u[0] == "v":
                    v_unit(u[1])
                else:
                    kq_unit(*u)

            # ---- projection (emitted lazily, one query block behind) ----
            def make_proj(qb):
                q0 = qb * QB

                def emit(co, y=None, phase="all"):
                    # phase "a": heads 0-3 only (available early); "b": the
                    # remainder.  Used to overlap the last block's projection
                    # with its epilogue chain.
                    if y is None:
                        y = opool.tile(
                            [128, QB], F32, tag="oacc", name=f"y{qb}_{co}"
                        )
                    cis = {"all": range(H), "a": range(4), "b": range(4, H)}[phase]
                    for ci in cis:
                        nc.tensor.matmul(
                            y,
                            wp_sb[:, ci, co * 128 : (co + 1) * 128],
                            at_sb[ci][:, q0 : q0 + QB],
                            start=(ci == 0),
                            stop=False,
                        )
                    if phase == "a":
                        return y
                    nc.tensor.matmul(
                        y,
                        bp_sb[:, co * 128 : (co + 1) * 128],
                        ones_sb,
                        start=False,
                        stop=True,
                    )
                    ysb = ypool.tile([128, QB], F32, tag="y", name=f"ysb{qb}_{co}")
                    nc.vector.tensor_copy(out=ysb, in_=y)
                    nc.sync.dma_start(out=out_t[co, :, q0 : q0 + QB], in_=ysb)

                return emit

            pending_proj = None

            # ---- attention: (query-512-block, head-pair) x 32 key tiles ----
            for qb in range(NQB):
                q0 = qb * QB
                for hp in range(3):
                    hA, hB = 2 * hp, 2 * hp + 1
                    oaccA = opool.tile([D + 1, QB], F32, tag="oacc", name=f"oaccA{qb}_{hp}")
                    oaccB = opool.tile([D + 1, QB], F32, tag="oacc", name=f"oaccB{qb}_{hp}")
                    pending = None
                    for k in range(NKT):
                        s = spool.tile([128, 2 * QB], F32, tag="s")
                        nc.tensor.matmul(
                            s[:, 0:QB],
                            kt_sb[hp][0:D, k * KT : (k + 1) * KT],
                            qt_sb[hp][0:D, q0 : q0 + QB],
                            start=True,
                            stop=True,
                        )
                        nc.tensor.matmul(
                            s[:, QB : 2 * QB],
                            kt_sb[hp][D : 2 * D, k * KT : (k + 1) * KT],
                            qt_sb[hp][D : 2 * D, q0 : q0 + QB],
                            start=True,
                            stop=True,
                        )
                        p = ppool.tile([128, 2 * QB], BF16, tag="p")
                        nc.scalar.activation(
                            p, s, mybir.ActivationFunctionType.Exp, scale=SCALE
                        )
                        if pending is not None:
                            pk, pp = pending
                            for oacc, h, o in ((oaccA, hA, 0), (oaccB, hB, QB)):
                                nc.tensor.matmul(
                                    oacc,
                                    vp_sb[pk][:, h * (D + 1) : (h + 1) * (D + 1)],
                                    pp[:, o : o + QB],
                                    start=(pk == 0),
                                    stop=False,
                                )
                        pending = (k, p)
                        # drain deferred work into the stream (PE has slack)
                        if units and len(units) > 32:
                            emit_unit(units.pop())
                            emit_unit(units.pop())
                        elif units and k % 2 == 1:
                            emit_unit(units.pop())
                        if pending_proj is not None and hp == 0 and k in (6, 11, 16):
                            pending_proj({6: 0, 11: 1, 16: 2}[k])
                            if k == 16:
                                pending_proj = None
                    pk, pp = pending
                    for oacc, h, o in ((oaccA, hA, 0), (oaccB, hB, QB)):
                        nc.tensor.matmul(
                            oacc,
                            vp_sb[pk][:, h * (D + 1) : (h + 1) * (D + 1)],
                            pp[:, o : o + QB],
                            start=False,
                            stop=True,
                        )
                    # normalize both heads: attn^T = oacc[0:D] * (1/Z).
                    # First copy the accumulator out of PSUM (frees the slot
                    # fast); the reciprocal + partition-broadcast (via a DRAM
                    # bounce) + multiply then run entirely on DVE/DMA, never
                    # blocking the in-order PE stream.
                    last = qb == NQB - 1 and hp == 2
                    for oacc, h in ((oaccA, hA), (oaccB, hB)):
                        if last:
                            # tail: PE is idle after the final AVs — broadcast
                            # 1/Z with a K=1 ones-matmul instead of the
                            # (higher-latency) DMA bounce.
                            r1 = zpool.tile(
                                [D + 1, QB], F32, tag="au", name=f"r1_{qb}_{h}"
                            )
                            nc.vector.reciprocal(
                                out=r1[D : D + 1, :], in_=oacc[D : D + 1, :]
                            )
                            rbp = opool.tile(
                                [D, QB], F32, tag="oacc", name=f"rbp{qb}_{h}"
                            )
                            nc.tensor.matmul(
                                rbp,
                                ones_f32[D : D + 1, :],
                                r1[D : D + 1, :],
                                start=True,
                                stop=True,
                            )
                            au2 = zpool.tile(
                                [D + 1, QB], F32, tag="au", name=f"au2_{qb}_{h}"
                            )
                            nc.vector.tensor_copy(out=au2[0:D, :], in_=oacc[0:D, :])
                            nc.vector.tensor_mul(
                                out=at_sb[h][:, q0 : q0 + QB],
                                in0=au2[0:D, :],
                                in1=rbp,
                            )
                            continue
                        au = zpool.tile([D + 1, QB], F32, tag="au", name=f"au{qb}_{h}")
                        nc.vector.tensor_copy(out=au, in_=oacc)
                        nc.vector.reciprocal(
                            out=au[D : D + 1, :], in_=au[D : D + 1, :]
                        )
                        rd = dpool.tile([1, QB], F32, tag="rd", name=f"rd{qb}_{h}")
                        nc.sync.dma_start(out=rd, in_=au[D : D + 1, :])
                        rb = zpool.tile([D, QB], F32, tag="rb", name=f"rb{qb}_{h}")
                        nc.sync.dma_start(out=rb, in_=rd.to_broadcast([D, QB]))
                        nc.vector.tensor_mul(
                            out=at_sb[h][:, q0 : q0 + QB], in0=au[0:D, :], in1=rb
                        )
                if qb < NQB - 1:
                    pending_proj = make_proj(qb)
            final_proj = make_proj(NQB - 1)
            for co in range(3):
                y = final_proj(co, phase="a")
                final_proj(co, y=y, phase="b")

    nc.compile()
    return nc


_NC_CACHE = {}


def _get_nc():
    if "nc" not in _NC_CACHE:
        _NC_CACHE["nc"] = _build()
    return _NC_CACHE["nc"]


def _prep_core_inputs(x, w_qkv, w_proj, b_proj):
    """Host-side sharding: returns in_maps for the 8 cores."""
    wqkv_p = np.ascontiguousarray(w_qkv.reshape(3, 128, 3 * C)).astype(BF16_NP)
    wproj_p = np.ascontiguousarray(w_proj.reshape(H, D, C)).astype(BF16_NP)
    bproj_p = np.ascontiguousarray(b_proj.reshape(1, C)).astype(BF16_NP)
    in_maps = []
    for core in range(NCORES):
        b, qh = core // 2, core % 2
        xt_b = np.ascontiguousarray(x[b].T).astype(BF16_NP)  # [C, N]
        xq_b = np.ascontiguousarray(x[b, qh * NQ : (qh + 1) * NQ].T).astype(BF16_NP)
        in_maps.append(
            {
                "xt": xt_b.reshape(3, 128, N),
                "xq": xq_b.reshape(3, 128, NQ),
                "wqkv": wqkv_p,
                "wproj": wproj_p,
                "bproj": bproj_p,
            }
        )
    return in_maps


def run(inputs, **kw):
    """Run the kernel; returns (full_output, BassKernelResults)."""
    x = np.asarray(inputs["x"], dtype=np.float32)
    w_qkv = np.asarray(inputs["w_qkv"], dtype=np.float32)
    w_proj = np.asarray(inputs["w_proj"], dtype=np.float32)
    b_proj = np.asarray(inputs["b_proj"], dtype=np.float32)

    nc = _get_nc()
    in_maps = _prep_core_inputs(x, w_qkv, w_proj, b_proj)
    res = run_bass_kernel_spmd(nc, in_maps, core_ids=list(range(NCORES)), **kw)

    out = np.empty((B, N, C), dtype=np.float32)
    for core in range(NCORES):
        b, qh = core // 2, core % 2
        yt = res.results[core]["out_t"].reshape(C, NQ)  # [3*128, NQ]
        out[b, qh * NQ : (qh + 1) * NQ, :] = yt.T
    return out, res


def kernel(**inputs) -> np.ndarray:
    out, _ = run(inputs)
    return out



# revision 3
# speedup vs baseline: 1.0255x; 1.0255x over previous
"""Multi-head self-attention block (B=4, N=4096, C=384, H=6, D=64) on 8 TRN2
NeuronCores.

Sharding: core c handles batch b = c // 2 and query rows
[(c % 2) * 2048, (c % 2 + 1) * 2048).  Fully data-parallel SPMD; no
collectives.

v2 over the original baseline (541us -> target ~400us):
  - exp split across TWO engines: ScalarE exact Exp (3/5 of key tiles) and
    DVE Schraudolph fast-exp (2/5): one tensor_scalar mult+add writing int16
    that bitcasts to fp16 (i = 184.665*s + 15301; zero-mean C=59 tuning;
    max rel err ~4% on 40% of weights -> ~1e-2 end-to-end, gate is 2e-2).
  - output projection contracts HEAD PAIRS (K=128 streams 2 cols/cycle vs
    K=64's 1 col/cycle): at tiles hold two heads stacked [128, NQ]; bias is
    folded into the PSUM->SBUF copy as a per-partition tensor_scalar add.
  - reciprocal of Z batched per query block into ONE [128, 24] DVE op (was
    24 single-partition [1,512] ops at ~3.3us each) via a DRAM bounce.
  - normalization multiplies moved to GpSimd (SBUF-only engine, idle).
  - attention accumulator copies (PSUM->SBUF) on DVE; odd heads shifted to
    partitions 64:128 of the pair tile by SBUF->SBUF DMA.
"""

import numpy as np

import concourse.bass as bass
import concourse.tile as tile
import concourse.mybir as mybir
from concourse import bacc
from concourse.bass_utils import run_bass_kernel_spmd

# Problem dims (hardcoded per contract)
B, N, C = 4, 4096, 384
H, D = 6, 64
SCALE = D**-0.5
NCORES = 8
NQ = N // 2  # queries per core
QB = 512  # query block (PSUM bank of AV accumulation per head)
NQB = NQ // QB  # 4
KT = 128  # key tile
NKT = N // KT  # 32
F32 = mybir.dt.float32
F16 = mybir.dt.float16
I16 = mybir.dt.int16
F16_NP = np.float16

# Schraudolph fast-exp constants for fp16 target: exp(SCALE*s) ~=
# bitcast_f16(int16(A*s + B)); A = 2^10*log2(e)*SCALE, B = 15*2^10 - 59.
EXP_A = (2.0**10) * 1.4426950408889634 * SCALE
EXP_B = 15.0 * (2.0**10) - 59.0
# key tile k uses ScalarE exact exp iff SCALAR_TILE[k % 5]
SCALAR_TILE = (True, False, True, False, True)


def _build():
    nc = bacc.Bacc(None, target_bir_lowering=False)

    xt = nc.dram_tensor("xt", [3, 128, N], F16, kind="ExternalInput")
    xq = nc.dram_tensor("xq", [3, 128, NQ], F16, kind="ExternalInput")
    wqkv = nc.dram_tensor("wqkv", [3, 128, 3 * C], F16, kind="ExternalInput")
    wproj = nc.dram_tensor("wproj", [3, 128, C], F16, kind="ExternalInput")
    bproj = nc.dram_tensor("bproj", [128, 3], F32, kind="ExternalInput")
    out_t = nc.dram_tensor("out_t", [3, 128, NQ], F32, kind="ExternalOutput")

    with tile.TileContext(nc) as tc:
        with (
            tc.tile_pool(name="persist", bufs=1) as persist,
            tc.tile_pool(name="ppool", bufs=4) as ppool,
            tc.tile_pool(name="zpool", bufs=3) as zpool,
            tc.tile_pool(name="apool", bufs=8) as apool,
            tc.tile_pool(name="ypool", bufs=3) as ypool,
            tc.tile_pool(name="dpool", bufs=2, space="DRAM") as dpool,
            tc.tile_pool(name="spool", bufs=2, space="PSUM") as spool,
            tc.tile_pool(name="opool", bufs=4, space="PSUM") as opool,
        ):
            # ---- persistent SBUF tensors ----
            xt_sb = [persist.tile([128, N], F16, tag=f"xt{c}", name=f"xt_sb{c}") for c in range(3)]
            xq_sb = [persist.tile([128, NQ], F16, tag=f"xq{c}", name=f"xq_sb{c}") for c in range(3)]
            wq_sb = persist.tile([128, 3, 3 * C], F16, tag="wq")
            wp_sb = persist.tile([128, 3, C], F16, tag="wp")
            bp_sb = persist.tile([128, 3], F32, tag="bp")
            kt_sb = [persist.tile([128, N], F16, tag=f"kt{j}", name=f"kt_sb{j}") for j in range(3)]
            qt_sb = [persist.tile([128, NQ], F16, tag=f"qt{j}", name=f"qt_sb{j}") for j in range(3)]
            vp_sb = [
                persist.tile([128, H * (D + 1)], F16, tag=f"vp{k}", name=f"vp_sb{k}")
                for k in range(NKT)
            ]
            at_sb = [persist.tile([128, NQ], F16, tag=f"at{j}", name=f"at_sb{j}") for j in range(3)]

            # ---- input DMAs, ordered so the first QK matmuls start ASAP ----
            for c in range(3):
                nc.sync.dma_start(out=wq_sb[:, c, 0 : 2 * C], in_=wqkv[c, :, 0 : 2 * C])
            for c in range(3):
                nc.scalar.dma_start(out=xt_sb[c][:, 0:1024], in_=xt[c, :, 0:1024])
            for c in range(3):
                nc.gpsimd.dma_start(out=xq_sb[c][:, 0:512], in_=xq[c, :, 0:512])
            for c in range(3):
                nc.sync.dma_start(out=wq_sb[:, c, 2 * C : 3 * C], in_=wqkv[c, :, 2 * C : 3 * C])
            for t in range(1, 4):
                for c in range(3):
                    nc.gpsimd.dma_start(
                        out=xt_sb[c][:, t * 1024 : (t + 1) * 1024],
                        in_=xt[c, :, t * 1024 : (t + 1) * 1024],
                    )
            for c in range(3):
                nc.gpsimd.dma_start(out=xq_sb[c][:, 512:NQ], in_=xq[c, :, 512:NQ])
            for hp in range(3):
                nc.gpsimd.dma_start(out=wp_sb[:, hp, :], in_=wproj[hp, :, :])
            nc.gpsimd.dma_start(out=bp_sb, in_=bproj[:, :])

            # ---- QKV projection units (share the 1-bank opool slots) ----
            def v_unit(k):
                vps = opool.tile([128, C], F32, tag="oacc", name=f"vps{k}")
                for c in range(3):
                    nc.tensor.matmul(
                        vps,
                        xt_sb[c][:, k * KT : (k + 1) * KT],
                        wq_sb[:, c, 2 * C : 3 * C],
                        start=(c == 0),
                        stop=(c == 2),
                    )
                dst = vp_sb[k].rearrange("p (h e) -> p h e", e=D + 1)
                nc.vector.tensor_copy(
                    out=dst[:, :, 0:D], in_=vps.rearrange("p (h d) -> p h d", d=D)
                )
                nc.vector.memset(dst[:, :, D : D + 1], 1.0)

            def kq_unit(kind, j, t):
                kq = opool.tile([128, 512], F32, tag="oacc", name=f"kq{kind}{j}_{t}")
                coff = C + j * 128 if kind == "k" else j * 128
                src = xt_sb if kind == "k" else xq_sb
                dstt = kt_sb[j] if kind == "k" else qt_sb[j]
                for c in range(3):
                    nc.tensor.matmul(
                        kq,
                        wq_sb[:, c, coff : coff + 128],
                        src[c][:, t * 512 : (t + 1) * 512],
                        start=(c == 0),
                        stop=(c == 2),
                    )
                nc.vector.tensor_copy(out=dstt[:, t * 512 : (t + 1) * 512], in_=kq)

            # upfront: only head pair 0's first K/Q tiles; ALL other QKV
            # work (including V) drains into the attention stream.  PE is
            # in-order, so emission order guarantees each AV sees its V tile.
            kq_unit("k", 0, 0)
            kq_unit("q", 0, 0)

            units = []
            ks = [("k", 0, t) for t in range(1, N // 512)]
            for k in range(NKT):
                units.append(("v", k, 0))
                if k % 2 == 1 and ks:
                    units.append(ks.pop(0))
            units.extend(ks)
            for j in (1, 2):
                units.append(("k", j, 0))
                units.append(("q", j, 0))
                for t in range(1, N // 512):
                    units.append(("k", j, t))
                for t in range(1, NQ // 512):
                    units.append(("q", j, t))
            for t in range(1, NQ // 512):
                units.append(("q", 0, t))
            units.reverse()  # pop() from the end

            def emit_unit(u):
                if u[0] == "v":
                    v_unit(u[1])
                else:
                    kq_unit(*u)

            # ---- deferred output projection (head-pair contraction K=128) ----
            def make_proj(qb):
                q0 = qb * QB

                def emit(co):
                    y = opool.tile([128, QB], F32, tag="oacc", name=f"y{qb}_{co}")
                    for hp in range(3):
                        nc.tensor.matmul(
                            y,
                            wp_sb[:, hp, co * 128 : (co + 1) * 128],
                            at_sb[hp][:, q0 : q0 + QB],
                            start=(hp == 0),
                            stop=(hp == 2),
                        )
                    ysb = ypool.tile([128, QB], F32, tag="y", name=f"ysb{qb}_{co}")
                    # fused bias add during the PSUM->SBUF copy
                    nc.vector.tensor_scalar(
                        ysb, y, bp_sb[:, co : co + 1], None, mybir.AluOpType.add
                    )
                    nc.sync.dma_start(out=out_t[co, :, q0 : q0 + QB], in_=ysb)

                return emit

            pending_proj = None

            # per-qb Z bounce buffers (DRAM) and the batched reciprocal
            def z_dram(qb):
                t = dpool.tile([1, 6 * QB], F32, tag="zd", name=f"zd{qb}")
                r = dpool.tile([1, 6 * QB], F32, tag="rd", name=f"rd{qb}")
                return t, r

            # ---- attention: (query-512-block, head-pair) x 32 key tiles ----
            for qb in range(NQB):
                q0 = qb * QB
                zd, rd = z_dram(qb)
                au_tiles = {}
                for hp in range(3):
                    hA, hB = 2 * hp, 2 * hp + 1
                    oaccA = opool.tile([D + 1, QB], F32, tag="oacc", name=f"oaccA{qb}_{hp}")
                    oaccB = opool.tile([D + 1, QB], F32, tag="oacc", name=f"oaccB{qb}_{hp}")
                    pending = None
                    for k in range(NKT):
                        s = spool.tile([128, 2 * QB], F32, tag="s")
                        nc.tensor.matmul(
                            s[:, 0:QB],
                            kt_sb[hp][0:D, k * KT : (k + 1) * KT],
                            qt_sb[hp][0:D, q0 : q0 + QB],
                            start=True,
                            stop=True,
                        )
                        nc.tensor.matmul(
                            s[:, QB : 2 * QB],
                            kt_sb[hp][D : 2 * D, k * KT : (k + 1) * KT],
                            qt_sb[hp][D : 2 * D, q0 : q0 + QB],
                            start=True,
                            stop=True,
                        )
                        p = ppool.tile([128, 2 * QB], F16, tag="p")
                        if SCALAR_TILE[k % 5]:
                            nc.scalar.activation(
                                p, s, mybir.ActivationFunctionType.Exp, scale=SCALE
                            )
                        else:
                            nc.vector.tensor_scalar(
                                p.bitcast(I16),
                                s,
                                EXP_A,
                                EXP_B,
                                mybir.AluOpType.mult,
                                mybir.AluOpType.add,
                            )
                        if pending is not None:
                            pk, pp = pending
                            for oacc, h, o in ((oaccA, hA, 0), (oaccB, hB, QB)):
                                nc.tensor.matmul(
                                    oacc,
                                    vp_sb[pk][:, h * (D + 1) : (h + 1) * (D + 1)],
                                    pp[:, o : o + QB],
                                    start=(pk == 0),
                                    stop=False,
                                )
                        pending = (k, p)
                        # drain deferred QKV work into the stream (PE slack)
                        if units and len(units) > 32:
                            emit_unit(units.pop())
                            emit_unit(units.pop())
                        elif units and k % 2 == 1:
                            emit_unit(units.pop())
                        if pending_proj is not None and hp == 0 and k in (8, 14, 20):
                            pending_proj({8: 0, 14: 1, 20: 2}[k])
                            if k == 20:
                                pending_proj = None
                    pk, pp = pending
                    for oacc, h, o in ((oaccA, hA, 0), (oaccB, hB, QB)):
                        nc.tensor.matmul(
                            oacc,
                            vp_sb[pk][:, h * (D + 1) : (h + 1) * (D + 1)],
                            pp[:, o : o + QB],
                            start=False,
                            stop=True,
                        )
                    # copy accumulators out of PSUM (frees banks); stage the
                    # Z rows into the per-qb DRAM bounce buffer
                    for oacc, h in ((oaccA, hA), (oaccB, hB)):
                        au = apool.tile([D + 1, QB], F32, tag="au", name=f"au{qb}_{h}")
                        nc.vector.tensor_copy(out=au, in_=oacc)
                        nc.sync.dma_start(
                            out=zd[0:1, h * QB : (h + 1) * QB], in_=au[D : D + 1, :]
                        )
                        au_tiles[h] = au
                # batched reciprocal of all 6 heads' Z: [1, 3072] DRAM ->
                # [128, 24] SBUF -> reciprocal -> back to DRAM
                zq = zpool.tile([128, 24], F32, tag="zq", name=f"zq{qb}")
                nc.sync.dma_start(out=zq, in_=zd.rearrange("o (p f) -> (o p) f", p=128))
                rz = zpool.tile([128, 24], F32, tag="rz", name=f"rz{qb}")
                nc.vector.reciprocal(out=rz, in_=zq)
                nc.sync.dma_start(out=rd.rearrange("o (p f) -> (o p) f", p=128), in_=rz)
                # normalize: at = au * (1/Z) broadcast, on GpSimd (SBUF-only)
                for h in range(H):
                    hp, odd = h // 2, h % 2
                    rb = zpool.tile([D, QB], F32, tag="rb", name=f"rb{qb}_{h}")
                    nc.sync.dma_start(
                        out=rb, in_=rd[0:1, h * QB : (h + 1) * QB].to_broadcast([D, QB])
                    )
                    if not odd:
                        nc.gpsimd.tensor_mul(
                            out=at_sb[hp][0:D, q0 : q0 + QB],
                            in0=au_tiles[h][0:D, :],
                            in1=rb,
                        )
                    else:
                        tmp = apool.tile([D, QB], F16, tag="atmp", name=f"atmp{qb}_{h}")
                        nc.gpsimd.tensor_mul(out=tmp, in0=au_tiles[h][0:D, :], in1=rb)
                        nc.sync.dma_start(
                            out=at_sb[hp][D : 2 * D, q0 : q0 + QB], in_=tmp
                        )
                if qb < NQB - 1:
                    pending_proj = make_proj(qb)
            final_proj = make_proj(NQB - 1)
            for co in range(3):
                final_proj(co)

    nc.compile()
    return nc


_NC_CACHE = {}


def _get_nc():
    if "nc" not in _NC_CACHE:
        _NC_CACHE["nc"] = _build()
    return _NC_CACHE["nc"]


def _prep_core_inputs(x, w_qkv, w_proj, b_proj):
    """Host-side sharding: returns in_maps for the 8 cores."""
    wqkv_p = np.ascontiguousarray(w_qkv.reshape(3, 128, 3 * C)).astype(F16_NP)
    wproj_p = np.ascontiguousarray(w_proj.reshape(3, 128, C)).astype(F16_NP)
    bproj_p = np.ascontiguousarray(b_proj.reshape(3, 128).T).astype(np.float32)
    in_maps = []
    for core in range(NCORES):
        b, qh = core // 2, core % 2
        xt_b = np.ascontiguousarray(x[b].T).astype(F16_NP)  # [C, N]
        xq_b = np.ascontiguousarray(x[b, qh * NQ : (qh + 1) * NQ].T).astype(F16_NP)
        in_maps.append(
            {
                "xt": xt_b.reshape(3, 128, N),
                "xq": xq_b.reshape(3, 128, NQ),
                "wqkv": wqkv_p,
                "wproj": wproj_p,
                "bproj": bproj_p,
            }
        )
    return in_maps


def run(inputs, **kw):
    """Run the kernel; returns (full_output, BassKernelResults)."""
    x = np.asarray(inputs["x"], dtype=np.float32)
    w_qkv = np.asarray(inputs["w_qkv"], dtype=np.float32)
    w_proj = np.asarray(inputs["w_proj"], dtype=np.float32)
    b_proj = np.asarray(inputs["b_proj"], dtype=np.float32)

    nc = _get_nc()
    in_maps = _prep_core_inputs(x, w_qkv, w_proj, b_proj)
    res = run_bass_kernel_spmd(nc, in_maps, core_ids=list(range(NCORES)), **kw)

    out = np.empty((B, N, C), dtype=np.float32)
    for core in range(NCORES):
        b, qh = core // 2, core % 2
        yt = res.results[core]["out_t"].reshape(C, NQ)  # [3*128, NQ]
        out[b, qh * NQ : (qh + 1) * NQ, :] = yt.T
    return out, res


def kernel(**inputs) -> np.ndarray:
    out, _ = run(inputs)
    return out


# revision 4
# speedup vs baseline: 1.1748x; 1.1456x over previous
"""Multi-head self-attention block (B=4, N=4096, C=384, H=6, D=64) on 8 TRN2
NeuronCores.

Sharding: core c handles batch b = c // 2 and query rows
[(c % 2) * 2048, (c % 2 + 1) * 2048).  Fully data-parallel SPMD; no
collectives.

v2 over the original baseline (541us -> target ~400us):
  - exp split across TWO engines: ScalarE exact Exp (3/5 of key tiles) and
    DVE Schraudolph fast-exp (2/5): one tensor_scalar mult+add writing int16
    that bitcasts to fp16 (i = 184.665*s + 15301; zero-mean C=59 tuning;
    max rel err ~4% on 40% of weights -> ~1e-2 end-to-end, gate is 2e-2).
  - output projection contracts HEAD PAIRS (K=128 streams 2 cols/cycle vs
    K=64's 1 col/cycle): at tiles hold two heads stacked [128, NQ]; bias is
    folded into the PSUM->SBUF copy as a per-partition tensor_scalar add.
  - reciprocal of Z batched per query block into ONE [128, 24] DVE op (was
    24 single-partition [1,512] ops at ~3.3us each) via a DRAM bounce.
  - normalization multiplies moved to GpSimd (SBUF-only engine, idle).
  - attention accumulator copies (PSUM->SBUF) on DVE; odd heads shifted to
    partitions 64:128 of the pair tile by SBUF->SBUF DMA.
"""

import numpy as np

import concourse.bass as bass
import concourse.tile as tile
import concourse.mybir as mybir
from concourse import bacc
from concourse.bass_utils import run_bass_kernel_spmd

# Problem dims (hardcoded per contract)
B, N, C = 4, 4096, 384
H, D = 6, 64
SCALE = D**-0.5
NCORES = 8
NQ = N // 2  # queries per core
QB = 512  # query block (PSUM bank of AV accumulation per head)
NQB = NQ // QB  # 4
KT = 128  # key tile
NKT = N // KT  # 32
F32 = mybir.dt.float32
F16 = mybir.dt.float16
I16 = mybir.dt.int16
F16_NP = np.float16

# Schraudolph fast-exp constants for fp16 target: exp(SCALE*s) ~=
# bitcast_f16(int16(A*s + B)); A = 2^10*log2(e)*SCALE, B = 15*2^10 - 59.
EXP_A = (2.0**10) * 1.4426950408889634 * SCALE
EXP_B = 15.0 * (2.0**10) - 59.0
# key tile k uses ScalarE exact exp iff SCALAR_TILE[k % 5]
SCALAR_TILE = (True, False, True, False, True)


def _build():
    nc = bacc.Bacc(None, target_bir_lowering=False)

    xt = nc.dram_tensor("xt", [3, 128, N], F16, kind="ExternalInput")
    xq = nc.dram_tensor("xq", [3, 128, NQ], F16, kind="ExternalInput")
    wqkv = nc.dram_tensor("wqkv", [3, 128, 3 * C], F16, kind="ExternalInput")
    wproj = nc.dram_tensor("wproj", [3, 128, C], F16, kind="ExternalInput")
    bproj = nc.dram_tensor("bproj", [128, 3], F32, kind="ExternalInput")
    out_t = nc.dram_tensor("out_t", [3, 128, NQ], F32, kind="ExternalOutput")

    with tile.TileContext(nc) as tc:
        with (
            tc.tile_pool(name="persist", bufs=1) as persist,
            tc.tile_pool(name="ppool", bufs=5) as ppool,
            tc.tile_pool(name="zpool", bufs=3) as zpool,
            tc.tile_pool(name="apool", bufs=8) as apool,
            tc.tile_pool(name="ypool", bufs=3) as ypool,
            tc.tile_pool(name="dpool", bufs=2, space="DRAM") as dpool,
            tc.tile_pool(name="spool", bufs=2, space="PSUM") as spool,
            tc.tile_pool(name="opool", bufs=4, space="PSUM") as opool,
        ):
            # ---- persistent SBUF tensors ----
            xt_sb = [persist.tile([128, N], F16, tag=f"xt{c}", name=f"xt_sb{c}") for c in range(3)]
            xq_sb = [persist.tile([128, NQ], F16, tag=f"xq{c}", name=f"xq_sb{c}") for c in range(3)]
            wq_sb = persist.tile([128, 3, 3 * C], F16, tag="wq")
            wp_sb = persist.tile([128, 3, C], F16, tag="wp")
            bp_sb = persist.tile([128, 3], F32, tag="bp")
            kt_sb = [persist.tile([128, N], F16, tag=f"kt{j}", name=f"kt_sb{j}") for j in range(3)]
            qt_sb = [persist.tile([128, NQ], F16, tag=f"qt{j}", name=f"qt_sb{j}") for j in range(3)]
            vp_sb = [
                persist.tile([128, H * (D + 1)], F16, tag=f"vp{k}", name=f"vp_sb{k}")
                for k in range(NKT)
            ]
            at_sb = [persist.tile([128, NQ], F16, tag=f"at{j}", name=f"at_sb{j}") for j in range(3)]

            # ---- input DMAs, ordered so the first QK matmuls start ASAP ----
            for c in range(3):
                nc.sync.dma_start(out=wq_sb[:, c, 0 : 2 * C], in_=wqkv[c, :, 0 : 2 * C])
            for c in range(3):
                nc.scalar.dma_start(out=xt_sb[c][:, 0:1024], in_=xt[c, :, 0:1024])
            for c in range(3):
                nc.gpsimd.dma_start(out=xq_sb[c][:, 0:512], in_=xq[c, :, 0:512])
            for c in range(3):
                nc.sync.dma_start(out=wq_sb[:, c, 2 * C : 3 * C], in_=wqkv[c, :, 2 * C : 3 * C])
            for t in range(1, 4):
                for c in range(3):
                    nc.gpsimd.dma_start(
                        out=xt_sb[c][:, t * 1024 : (t + 1) * 1024],
                        in_=xt[c, :, t * 1024 : (t + 1) * 1024],
                    )
            for c in range(3):
                nc.gpsimd.dma_start(out=xq_sb[c][:, 512:NQ], in_=xq[c, :, 512:NQ])
            for hp in range(3):
                nc.gpsimd.dma_start(out=wp_sb[:, hp, :], in_=wproj[hp, :, :])
            nc.gpsimd.dma_start(out=bp_sb, in_=bproj[:, :])

            # ---- QKV projection units (share the 1-bank opool slots) ----
            def v_unit(k):
                vps = opool.tile([128, C], F32, tag="oacc", name=f"vps{k}")
                for c in range(3):
                    nc.tensor.matmul(
                        vps,
                        xt_sb[c][:, k * KT : (k + 1) * KT],
                        wq_sb[:, c, 2 * C : 3 * C],
                        start=(c == 0),
                        stop=(c == 2),
                    )
                dst = vp_sb[k].rearrange("p (h e) -> p h e", e=D + 1)
                nc.vector.tensor_copy(
                    out=dst[:, :, 0:D], in_=vps.rearrange("p (h d) -> p h d", d=D)
                )
                nc.vector.memset(dst[:, :, D : D + 1], 1.0)

            def kq_unit(kind, j, t):
                kq = opool.tile([128, 512], F32, tag="oacc", name=f"kq{kind}{j}_{t}")
                coff = C + j * 128 if kind == "k" else j * 128
                src = xt_sb if kind == "k" else xq_sb
                dstt = kt_sb[j] if kind == "k" else qt_sb[j]
                for c in range(3):
                    nc.tensor.matmul(
                        kq,
                        wq_sb[:, c, coff : coff + 128],
                        src[c][:, t * 512 : (t + 1) * 512],
                        start=(c == 0),
                        stop=(c == 2),
                    )
                nc.vector.tensor_copy(out=dstt[:, t * 512 : (t + 1) * 512], in_=kq)

            # upfront: only head pair 0's first K/Q tiles; ALL other QKV
            # work (including V) drains into the attention stream.  PE is
            # in-order, so emission order guarantees each AV sees its V tile.
            kq_unit("k", 0, 0)
            kq_unit("q", 0, 0)

            units = []
            ks = [("k", 0, t) for t in range(1, N // 512)]
            for k in range(NKT):
                units.append(("v", k, 0))
                if k % 2 == 1 and ks:
                    units.append(ks.pop(0))
            units.extend(ks)
            for j in (1, 2):
                units.append(("k", j, 0))
                units.append(("q", j, 0))
                for t in range(1, N // 512):
                    units.append(("k", j, t))
                for t in range(1, NQ // 512):
                    units.append(("q", j, t))
            for t in range(1, NQ // 512):
                units.append(("q", 0, t))
            units.reverse()  # pop() from the end

            def emit_unit(u):
                if u[0] == "v":
                    v_unit(u[1])
                else:
                    kq_unit(*u)

            # ---- deferred output projection (head-pair contraction K=128) ----
            def make_proj(qb):
                q0 = qb * QB

                def emit(co):
                    y = opool.tile([128, QB], F32, tag="oacc", name=f"y{qb}_{co}")
                    for hp in range(3):
                        nc.tensor.matmul(
                            y,
                            wp_sb[:, hp, co * 128 : (co + 1) * 128],
                            at_sb[hp][:, q0 : q0 + QB],
                            start=(hp == 0),
                            stop=(hp == 2),
                        )
                    ysb = ypool.tile([128, QB], F32, tag="y", name=f"ysb{qb}_{co}")
                    # fused bias add during the PSUM->SBUF copy
                    nc.vector.tensor_scalar(
                        ysb, y, bp_sb[:, co : co + 1], None, mybir.AluOpType.add
                    )
                    nc.sync.dma_start(out=out_t[co, :, q0 : q0 + QB], in_=ysb)

                return emit

            pending_proj = None

            # per-qb Z bounce buffers (DRAM) and the batched reciprocal
            def z_dram(qb):
                t = dpool.tile([1, 6 * QB], F32, tag="zd", name=f"zd{qb}")
                r = dpool.tile([1, 6 * QB], F32, tag="rd", name=f"rd{qb}")
                return t, r

            # ---- attention: (query-512-block, head-pair) x 32 key tiles ----
            for qb in range(NQB):
                q0 = qb * QB
                zd, rd = z_dram(qb)
                au_tiles = {}
                for hp in range(3):
                    hA, hB = 2 * hp, 2 * hp + 1
                    oaccA = opool.tile([D + 1, QB], F32, tag="oacc", name=f"oaccA{qb}_{hp}")
                    oaccB = opool.tile([D + 1, QB], F32, tag="oacc", name=f"oaccB{qb}_{hp}")
                    pendings = []
                    for k in range(NKT):
                        s = spool.tile([128, 2 * QB], F32, tag="s")
                        nc.tensor.matmul(
                            s[:, 0:QB],
                            kt_sb[hp][0:D, k * KT : (k + 1) * KT],
                            qt_sb[hp][0:D, q0 : q0 + QB],
                            start=True,
                            stop=True,
                        )
                        nc.tensor.matmul(
                            s[:, QB : 2 * QB],
                            kt_sb[hp][D : 2 * D, k * KT : (k + 1) * KT],
                            qt_sb[hp][D : 2 * D, q0 : q0 + QB],
                            start=True,
                            stop=True,
                        )
                        p = ppool.tile([128, 2 * QB], F16, tag="p")
                        if SCALAR_TILE[k % 5]:
                            nc.scalar.activation(
                                p, s, mybir.ActivationFunctionType.Exp, scale=SCALE
                            )
                        else:
                            nc.vector.tensor_scalar(
                                p.bitcast(I16),
                                s,
                                EXP_A,
                                EXP_B,
                                mybir.AluOpType.mult,
                                mybir.AluOpType.add,
                            )
                        if len(pendings) >= 2:
                            pk, pp = pendings.pop(0)
                            for oacc, h, o in ((oaccA, hA, 0), (oaccB, hB, QB)):
                                nc.tensor.matmul(
                                    oacc,
                                    vp_sb[pk][:, h * (D + 1) : (h + 1) * (D + 1)],
                                    pp[:, o : o + QB],
                                    start=(pk == 0),
                                    stop=False,
                                )
                        pendings.append((k, p))
                        # drain deferred QKV work into the stream (PE slack)
                        if units and len(units) > 32:
                            emit_unit(units.pop())
                            emit_unit(units.pop())
                        elif units and k % 2 == 1:
                            emit_unit(units.pop())
                        if pending_proj is not None and hp == 0 and k in (8, 14, 20):
                            pending_proj({8: 0, 14: 1, 20: 2}[k])
                            if k == 20:
                                pending_proj = None
                    for pk, pp in pendings:
                        for oacc, h, o in ((oaccA, hA, 0), (oaccB, hB, QB)):
                            nc.tensor.matmul(
                                oacc,
                                vp_sb[pk][:, h * (D + 1) : (h + 1) * (D + 1)],
                                pp[:, o : o + QB],
                                start=(pk == 0),
                                stop=(pk == NKT - 1),
                            )
                    # copy accumulators out of PSUM (frees banks); stage the
                    # Z rows into the per-qb DRAM bounce buffer
                    for oacc, h in ((oaccA, hA), (oaccB, hB)):
                        au = apool.tile([D + 1, QB], F32, tag="au", name=f"au{qb}_{h}")
                        nc.vector.tensor_copy(out=au, in_=oacc)
                        nc.sync.dma_start(
                            out=zd[0:1, h * QB : (h + 1) * QB], in_=au[D : D + 1, :]
                        )
                        au_tiles[h] = au
                # batched reciprocal of all 6 heads' Z: [1, 3072] DRAM ->
                # [128, 24] SBUF -> reciprocal -> back to DRAM
                zq = zpool.tile([128, 24], F32, tag="zq", name=f"zq{qb}")
                nc.sync.dma_start(out=zq, in_=zd.rearrange("o (p f) -> (o p) f", p=128))
                rz = zpool.tile([128, 24], F32, tag="rz", name=f"rz{qb}")
                nc.vector.reciprocal(out=rz, in_=zq)
                nc.sync.dma_start(out=rd.rearrange("o (p f) -> (o p) f", p=128), in_=rz)
                # normalize: at = au * (1/Z) broadcast, on GpSimd (SBUF-only)
                for h in range(H):
                    hp, odd = h // 2, h % 2
                    rb = zpool.tile([D, QB], F32, tag="rb", name=f"rb{qb}_{h}")
                    nc.sync.dma_start(
                        out=rb, in_=rd[0:1, h * QB : (h + 1) * QB].to_broadcast([D, QB])
                    )
                    if not odd:
                        nc.gpsimd.tensor_mul(
                            out=at_sb[hp][0:D, q0 : q0 + QB],
                            in0=au_tiles[h][0:D, :],
                            in1=rb,
                        )
                    else:
                        tmp = apool.tile([D, QB], F16, tag="atmp", name=f"atmp{qb}_{h}")
                        nc.gpsimd.tensor_mul(out=tmp, in0=au_tiles[h][0:D, :], in1=rb)
                        nc.sync.dma_start(
                            out=at_sb[hp][D : 2 * D, q0 : q0 + QB], in_=tmp
                        )
                if qb < NQB - 1:
                    pending_proj = make_proj(qb)
            final_proj = make_proj(NQB - 1)
            for co in range(3):
                final_proj(co)

    nc.compile()
    return nc


_NC_CACHE = {}


def _get_nc():
    if "nc" not in _NC_CACHE:
        _NC_CACHE["nc"] = _build()
    return _NC_CACHE["nc"]


def _prep_core_inputs(x, w_qkv, w_proj, b_proj):
    """Host-side sharding: returns in_maps for the 8 cores."""
    wqkv_p = np.ascontiguousarray(w_qkv.reshape(3, 128, 3 * C)).astype(F16_NP)
    wproj_p = np.ascontiguousarray(w_proj.reshape(3, 128, C)).astype(F16_NP)
    bproj_p = np.ascontiguousarray(b_proj.reshape(3, 128).T).astype(np.float32)
    in_maps = []
    for core in range(NCORES):
        b, qh = core // 2, core % 2
        xt_b = np.ascontiguousarray(x[b].T).astype(F16_NP)  # [C, N]
        xq_b = np.ascontiguousarray(x[b, qh * NQ : (qh + 1) * NQ].T).astype(F16_NP)
        in_maps.append(
            {
                "xt": xt_b.reshape(3, 128, N),
                "xq": xq_b.reshape(3, 128, NQ),
                "wqkv": wqkv_p,
                "wproj": wproj_p,
                "bproj": bproj_p,
            }
        )
    return in_maps


def run(inputs, **kw):
    """Run the kernel; returns (full_output, BassKernelResults)."""
    x = np.asarray(inputs["x"], dtype=np.float32)
    w_qkv = np.asarray(inputs["w_qkv"], dtype=np.float32)
    w_proj = np.asarray(inputs["w_proj"], dtype=np.float32)
    b_proj = np.asarray(inputs["b_proj"], dtype=np.float32)

    nc = _get_nc()
    in_maps = _prep_core_inputs(x, w_qkv, w_proj, b_proj)
    res = run_bass_kernel_spmd(nc, in_maps, core_ids=list(range(NCORES)), **kw)

    out = np.empty((B, N, C), dtype=np.float32)
    for core in range(NCORES):
        b, qh = core // 2, core % 2
        yt = res.results[core]["out_t"].reshape(C, NQ)  # [3*128, NQ]
        out[b, qh * NQ : (qh + 1) * NQ, :] = yt.T
    return out, res


def kernel(**inputs) -> np.ndarray:
    out, _ = run(inputs)
    return out
